# revision 25
# baseline (speedup 1.0000x reference)
"""Trainium2 Bass kernel for GQA attention (b=2, s=2048, dim=1024, 16 q / 4 kv heads).

Sharding: 8 cores = 2 (batch) x 4 (head groups). Each core owns one batch
element and 4 q-heads + 1 kv-head (Wq/Wk/Wv column-sharded, Wo row-sharded).
Host sums the 4 Wo partials per batch element.

On-device dataflow (per core), everything contraction-major:
  1. qkvT = Wqkv_g @ X^T          (bf16 matmuls, fp32 PSUM)
  2. RoPE on qT/kT via a pair-swap permutation matmul + DVE elementwise
  3. attention with transposed scores S^T[tk, tq] = K_roped @ Q_roped^T:
     exp on ScalarE (scale=1/8 folded in), softmax denominator Z via packed
     M=1 ones-matmuls, P@V with V as the stationary operand (col-packed per
     head pair), normalization by 1/Z applied via gpsimd partition_broadcast
     + DVE multiply on the small attn^T output.
  4. O_partial = attn_g^T.T @ Wo_g^T   -> DRAM fp32
"""

import sys
from contextlib import ExitStack

for _p in ("/opt/trn_rl_repo",):
    if _p not in sys.path:
        sys.path.insert(0, _p)

import numpy as np
import ml_dtypes

BF16 = ml_dtypes.bfloat16

P = 128
S = 2048          # sequence length
DIM = 1024        # model dim
HD = 64           # head dim
NT = S // P       # 16 token tiles
N_CT = DIM // P   # 8 contraction tiles for qkv proj
QKV = 384         # per-core projection rows: 256 q + 64 k + 64 v
PHW = 1024        # tq phase width
NPH = S // PHW    # 2 phases

_NC_CACHE = {}


def _build_kernel_program():
    import concourse.bass as bass
    import concourse.tile as tile
    from concourse import bacc, mybir

    dt = mybir.dt
    f32, bf16 = dt.float32, dt.bfloat16
    AF = mybir.ActivationFunctionType

    nc = bacc.Bacc("TRN2", target_bir_lowering=False, debug=False)

    xt = nc.dram_tensor("xt", [DIM, S], bf16, kind="ExternalInput").ap()
    wqkv = nc.dram_tensor("wqkv", [DIM, QKV], bf16, kind="ExternalInput").ap()
    wo = nc.dram_tensor("wo", [256, DIM], bf16, kind="ExternalInput").ap()
    cost = nc.dram_tensor("cost", [P, S], bf16, kind="ExternalInput").ap()
    sint = nc.dram_tensor("sint", [P, S], bf16, kind="ExternalInput").ap()
    rott = nc.dram_tensor("rott", [P, P], bf16, kind="ExternalInput").ap()
    ident = nc.dram_tensor("ident", [P, P], bf16, kind="ExternalInput").ap()
    utm = nc.dram_tensor("utm", [P, P], bf16, kind="ExternalInput").ap()
    identf = nc.dram_tensor("identf", [P, P], f32, kind="ExternalInput").ap()
    ida = nc.dram_tensor("ida", [P, P], bf16, kind="ExternalInput").ap()
    outp = nc.dram_tensor("outp", [S, DIM], bf16, kind="ExternalOutput").ap()

    with tile.TileContext(nc) as tc:
        with ExitStack() as ctx:
            _emit(ctx, tc, nc, mybir, bass, dict(
                xt=xt, wqkv=wqkv, wo=wo, cost=cost, sint=sint, rott=rott,
                ident=ident, utm=utm, identf=identf, ida=ida, outp=outp,
            ), f32, bf16, AF)
    nc.compile()
    return nc


def _emit(ctx, tc, nc, mybir, bass, io, f32, bf16, AF):
    tp = tc.tile_pool

    const = ctx.enter_context(tp(name="const", bufs=1))
    persist = ctx.enter_context(tp(name="persist", bufs=1))
    tmp = ctx.enter_context(tp(name="tmp", bufs=4))
    ptp = ctx.enter_context(tp(name="pt", bufs=8))
    big = ctx.enter_context(tp(name="big", bufs=2, space="PSUM"))      # [128,1024] slots
    pvp = ctx.enter_context(tp(name="pv", bufs=4, space="PSUM"))       # [128,512] slots

    # ---- weights + activations in (issue order = need order) ----
    wqkv_sb = persist.tile([P, N_CT, QKV], bf16, name="wqkv_sb", tag="wqkv_sb")
    nc.sync.dma_start(wqkv_sb[:], io["wqkv"].rearrange("(a p) d -> p a d", p=P))
    xt_sb = persist.tile([P, N_CT, S], bf16, name="xt_sb", tag="xt_sb")
    xt_r = io["xt"].rearrange("(a p) t -> p a t", p=P)
    for a in range(N_CT):
        nc.sync.dma_start(xt_sb[:, a, :], xt_r[:, a, :])
    rott_sb = const.tile([P, P], bf16, tag="rott")
    nc.sync.dma_start(rott_sb[:], io["rott"])
    ident_sb = const.tile([P, P], bf16, tag="ident")
    nc.sync.dma_start(ident_sb[:], io["ident"])
    utm_sb = const.tile([P, P], bf16, tag="utm")
    nc.sync.dma_start(utm_sb[:], io["utm"])
    identf_sb = const.tile([P, P], f32, tag="identf")
    nc.sync.dma_start(identf_sb[:], io["identf"])
    ida_sb = const.tile([P, P], bf16, tag="ida")
    nc.sync.dma_start(ida_sb[:], io["ida"])
    cost_sb = persist.tile([P, S], bf16, name="cost_sb", tag="cost_sb")
    nc.sync.dma_start(cost_sb[:], io["cost"])
    sint_sb = persist.tile([P, S], bf16, name="sint_sb", tag="sint_sb")
    nc.sync.dma_start(sint_sb[:], io["sint"])
    wo_sb = persist.tile([P, 2, DIM], bf16, name="wo_sb", tag="wo_sb")
    nc.sync.dma_start(wo_sb[:], io["wo"].rearrange("(a p) e -> p a e", p=P))

    # ---- phase 1: qkv projection ----
    # qkvT tiles (bf16): q01T [128,S] (heads 0,1), q23T (heads 2,3),
    # kvT [128,S] (rows 0:64 kT, 64:128 vT)
    q01T = persist.tile([P, S], bf16, name="q01T", tag="q01T")
    q23T = persist.tile([P, S], bf16, name="q23T", tag="q23T")
    kvT = persist.tile([P, S], bf16, name="kvT", tag="kvT")
    qkv_dst = [q01T, q23T, kvT]
    for mt in (0, 2):
        for n in range(S // 512):
            ps = big.tile([P, 1024], f32, name="ps", tag="big")[:, :512]
            for cti in range(N_CT):
                nc.tensor.matmul(
                    ps,
                    wqkv_sb[:, cti, mt * P:(mt + 1) * P],
                    xt_sb[:, cti, n * 512:(n + 1) * 512],
                    start=(cti == 0), stop=(cti == N_CT - 1),
                )
            nc.scalar.copy(qkv_dst[mt][:, n * 512:(n + 1) * 512], ps)

    # ---- phase 2: RoPE (on q heads and k) ----
    # roped out: q01r, q23r bf16; krep rows 0:64 (k roped), rows 64:128 copy
    q01r = persist.tile([P, S], bf16, name="q01r", tag="q01r")
    q23r = persist.tile([P, S], bf16, name="q23r", tag="q23r")
    krep = persist.tile([P, S], bf16, name="krep", tag="krep")
    rope_jobs = [(q01T, q01r, P), (kvT, krep, 64)]
    for src, dst, rows in rope_jobs:
        for n in range(S // 512):
            sl = slice(n * 512, (n + 1) * 512)
            psr = big.tile([P, 1024], f32, name="psr", tag="big")[:rows, :512]
            nc.tensor.matmul(
                psr, rott_sb[:rows, :rows], src[:rows, sl],
                start=True, stop=True,
            )
            t1 = tmp.tile([P, 512], bf16, name="ropet1", tag="rope")[:rows]
            nc.vector.tensor_mul(t1, src[:rows, sl], cost_sb[:rows, sl])
            t2 = tmp.tile([P, 512], bf16, name="ropet2", tag="rope")[:rows]
            nc.vector.tensor_mul(t2, psr, sint_sb[:rows, sl])
            nc.vector.tensor_add(dst[:rows, sl], t1, t2)
    # replicate roped k to partitions 64:128 for row-packed score matmuls
    nc.sync.dma_start(krep[64:128, :], krep[0:64, :])

    # ---- phase 3: V = (vT)^T, tiled [tk-tile][128, 64] ----
    v_sb = persist.tile([P, NT, HD + 1], bf16, name="v_sb", tag="v_sb")
    nc.vector.memset(v_sb[:, :, 0:1], 1.0)

    def fill_v_tile(j):
        pst = big.tile([P, 2048], bf16, name="pst", tag="big")[:, :HD]
        nc.tensor.transpose(pst, kvT[64:128, j * P:(j + 1) * P], ident_sb[64:128, 0:HD])
        nc.scalar.copy(v_sb[:, j, 1:HD + 1], pst)

    for j in range(NT // 2):
        fill_v_tile(j)

    # ---- phase 4: attention ----
    # attnT: [4*64 d-rows, S]: two tiles (heads 0,1 | heads 2,3)
    attnT = [persist.tile([P, S], bf16, name="attnT01", tag="attnT01"),
             persist.tile([P, S], bf16, name="attnT23", tag="attnT23")]
    qrs = [q01r, q23r]

    fill_state = {}

    def fill_q23_proj(j):
        # deferred q23 projection, two j-steps per 512-chunk
        n, half = j // 2, j % 2
        if n >= S // 512:
            return
        if half == 0:
            fill_state[n] = big.tile([P, 1024], f32, name="psq23", tag="big")[:, :512]
        psn = fill_state[n]
        for cti in (range(0, 4) if half == 0 else range(4, 8)):
            nc.tensor.matmul(
                psn, wqkv_sb[:, cti, P:2 * P],
                xt_sb[:, cti, n * 512:(n + 1) * 512],
                start=(cti == 0), stop=(cti == N_CT - 1),
            )
        if half == 1:
            nc.scalar.copy(q23T[:, n * 512:(n + 1) * 512], psn)
            del fill_state[n]

    def fill_rope_q23():
        for n in range(S // 512):
            sl = slice(n * 512, (n + 1) * 512)
            psr = big.tile([P, 1024], f32, name="psrf", tag="big")[:, :512]
            nc.tensor.matmul(psr, rott_sb[:], q23T[:, sl], start=True, stop=True)
            t1 = tmp.tile([P, 512], bf16, name="ropet1", tag="rope")
            nc.vector.tensor_mul(t1, q23T[:, sl], cost_sb[:, sl])
            t2 = tmp.tile([P, 512], bf16, name="ropet2", tag="rope")
            nc.vector.tensor_mul(t2, psr, sint_sb[:, sl])
            nc.vector.tensor_add(q23r[:, sl], t1, t2)

    def emit_wo(tt, e):
        if e == 0:
            fill_state["osb"] = tmp.tile([P, DIM], bf16, name="osb", tag="osb", bufs=2)
        osb = fill_state["osb"]
        po = big.tile([P, 1024], f32, name="po", tag="big")[:, :512]
        nc.tensor.matmul(
            po, attnT[0][:, tt * P:(tt + 1) * P], wo_sb[:, 0, e * 512:(e + 1) * 512],
            start=True, stop=False,
        )
        nc.tensor.matmul(
            po, attnT[1][:, tt * P:(tt + 1) * P], wo_sb[:, 1, e * 512:(e + 1) * 512],
            start=False, stop=True,
        )
        nc.vector.tensor_copy(osb[:, e * 512:(e + 1) * 512], po)
        if e == 1:
            nc.sync.dma_start(io["outp"][tt * P:(tt + 1) * P, :], osb[:])

    for ph in range(NPH):
        base = ph * PHW
        jmax = (base + PHW) // P - 1  # last tk tile in this phase
        for hp in range(2):
            # per-chunk PV accumulators; rows 0:64 attn^T, row 64 = Z (V ones col)
            pvE = [pvp.tile([P, 512], f32, name="pvE", tag="pv") for _ in range(2)]
            pvO = [pvp.tile([P, 512], f32, name="pvO", tag="pv") for _ in range(2)]
            for j in range(jmax + 1):
                ssl = max(0, j * P - base)    # phase-local valid tq start
                # --- scores S^T pair (span psum) + single span exp ---
                ptA = ptp.tile([P, PHW], bf16, name="ptA", tag="pt")
                ptB = ptp.tile([P, PHW], bf16, name="ptB", tag="pt")
                sA = big.tile([P, 1024], f32, name="sA", tag="big")
                sB = big.tile([P, 1024], f32, name="sB", tag="big")
                pieces = []
                if ssl < 512:
                    pieces.append((ssl, 512))
                pieces.append((max(512, ssl), PHW))
                for (lo, hi) in pieces:
                    if lo >= hi:
                        continue
                    nc.tensor.matmul(
                        sA[:, lo:hi],
                        krep[0:64, j * P:(j + 1) * P],
                        qrs[hp][0:64, base + lo:base + hi],
                        start=True, stop=True, tile_position=(0, 0),
                    )
                    nc.tensor.matmul(
                        sB[:, lo:hi],
                        krep[64:128, j * P:(j + 1) * P],
                        qrs[hp][64:128, base + lo:base + hi],
                        start=True, stop=True, tile_position=(64, 0),
                    )
                nc.scalar.activation(ptA[:, ssl:], sA[:, ssl:], AF.Exp, scale=0.125)
                nc.scalar.activation(ptB[:, ssl:], sB[:, ssl:], AF.Exp, scale=0.125)
                # --- causal mask on the diagonal 128-block ---
                if j * P >= base:
                    dl = slice(ssl, ssl + P)
                    nc.vector.tensor_mul(ptA[:, dl], ptA[:, dl], utm_sb[:])
                    nc.vector.tensor_mul(ptB[:, dl], ptB[:, dl], utm_sb[:])
                # --- PV (fused denominator row); evict chunk when complete ---
                for cl in range(2):
                    lo = max(512 * cl, ssl)
                    hi = 512 * (cl + 1)
                    if lo >= hi:
                        continue
                    c_glob = 2 * ph + cl
                    stop_j = min(4 * c_glob + 3, jmax)
                    st = (j == 0)
                    sp = (j == stop_j)
                    co = slice(lo - 512 * cl, hi - 512 * cl)
                    nc.tensor.matmul(
                        pvE[cl][0:HD + 1, co], v_sb[:, j, :], ptA[:, lo:hi],
                        start=st, stop=sp,
                    )
                    nc.tensor.matmul(
                        pvO[cl][0:HD + 1, co], v_sb[:, j, :], ptB[:, lo:hi],
                        start=st, stop=sp,
                    )
                    if sp:
                        # chunk complete: transpose-normalize-transpose-back.
                        gcol = base + 512 * cl
                        pvs_sb = tmp.tile([P, 2, 512], f32, name="pvs", tag="pvs", bufs=2)
                        rec4 = tmp.tile([P, 2, 4], f32, name="rec4", tag="rec4", bufs=2)
                        anrm = tmp.tile([P, 2, 4, HD], bf16, name="anrm", tag="anrm", bufs=2)
                        for h, pv in ((0, pvE[cl]), (1, pvO[cl])):
                            nc.vector.tensor_copy(pvs_sb[0:HD + 1, h, :], pv[0:HD + 1, :])
                            pT = pvp.tile([P, 512], f32, name="pT", tag="pv")[:, 0:260]
                            for b in range(4):
                                nc.tensor.transpose(
                                    pT[:, 65 * b:65 * (b + 1)],
                                    pvs_sb[0:HD + 1, h, 128 * b:128 * (b + 1)],
                                    identf_sb[0:HD + 1, 0:HD + 1])
                            pT3 = pT.rearrange("p (b c) -> p b c", c=65)
                            nc.vector.reciprocal(rec4[:, h, :], pT3[:, :, 0])
                            nc.vector.tensor_mul(
                                anrm[:, h], pT3[:, :, 1:HD + 1],
                                rec4[:, h, :, None].broadcast_to([P, 4, HD]))
                        pout = pvp.tile([P, 512], bf16, name="pout", tag="pv")
                        for h in range(2):
                            tpos = (0, 0) if h == 0 else (0, HD)
                            rows = slice(0, HD) if h == 0 else slice(HD, P)
                            for b in range(4):
                                nc.tensor.transpose(
                                    pout[rows, 128 * b:128 * (b + 1)],
                                    anrm[:, h, b, :], ida_sb[:, :],
                                    tile_position=tpos)
                        nc.vector.tensor_copy(attnT[hp][:, gcol:gcol + 512], pout[:, 0:512])
                # interleaved filler work keeps the PE dense (clock stays warm)
                if ph == 0 and hp == 0:
                    fill_q23_proj(j)
                elif ph == 0 and hp == 1:
                    fill_v_tile(NT // 2 + j)
                elif ph == 1 and hp == 0:
                    emit_wo(j // 2, j % 2)
                elif ph == 1 and hp == 1 and j >= 12:
                    emit_wo(8 + (j - 12), 0)
                    emit_wo(8 + (j - 12), 1)
            if ph == 0 and hp == 0:
                fill_rope_q23()

    # ---- phase 5: remaining output projection tiles ----
    for tt in range(12, NT):
        for e in range(2):
            emit_wo(tt, e)


def _host_inputs(X, cos, sin, Wq, Wk, Wv, Wo):
    """Build the 8 per-core input maps (host-side sharding + layout prep)."""
    cosT = np.ascontiguousarray(cos.T)  # [64, 2048]
    sinT = np.ascontiguousarray(sin.T)
    cost = np.concatenate([cosT, cosT], 0).astype(BF16)  # [128, 2048]
    sint = np.concatenate([sinT, sinT], 0).astype(BF16)
    rott = np.zeros((P, P), np.float32)
    idx = np.arange(0, P, 2)
    rott[idx, idx + 1] = 1.0    # RT[2i, 2i+1] = +1
    rott[idx + 1, idx] = -1.0   # RT[2i+1, 2i] = -1
    rott = rott.astype(BF16)
    ident = np.zeros((P, P), np.float32)
    ident[0:64, 0:64] = np.eye(64)
    ident[64:128, 0:64] = np.eye(64)   # same I64 available at base partition 64
    ident = ident.astype(BF16)
    utm = np.triu(np.ones((P, P), np.float32)).astype(BF16)
    identf = np.eye(P, dtype=np.float32)
    ida = np.eye(P, dtype=np.float32).astype(BF16)

    xts = [np.ascontiguousarray(X[b].T).astype(BF16) for b in range(X.shape[0])]

    in_maps = []
    for c in range(8):
        b, g = c // 4, c % 4
        wqkv = np.concatenate(
            [Wq[256 * g:256 * (g + 1)], Wk[64 * g:64 * (g + 1)], Wv[64 * g:64 * (g + 1)]], 0
        ).T.astype(BF16)                                   # [1024, 384]
        wog = np.ascontiguousarray(Wo[:, 256 * g:256 * (g + 1)].T).astype(BF16)  # [256, 1024]
        in_maps.append({
            "xt": xts[b], "wqkv": np.ascontiguousarray(wqkv), "wo": wog,
            "cost": cost, "sint": sint, "rott": rott, "ident": ident,
            "utm": utm, "identf": identf, "ida": ida,
        })
    return in_maps


def get_nc():
    if "nc" not in _NC_CACHE:
        _NC_CACHE["nc"] = _build_kernel_program()
    return _NC_CACHE["nc"]


def _install_ntff_hook():
    """The agent image's antenv lacks axon_hooks; recreate it so trace=True
    can reach the terminal's NRT profiler (timing only, not needed for
    correctness)."""
    import types
    if "antenv.axon_hooks" in sys.modules:
        return
    try:
        import antenv
        m = types.ModuleType("antenv.axon_hooks")
        holder = {"v": None}
        m.set_axon_ntff_profile_hook = lambda h: holder.__setitem__("v", h)
        m.get_axon_ntff_profile_hook = lambda: holder["v"]
        sys.modules["antenv.axon_hooks"] = m
        antenv.axon_hooks = m
        from trn_agent_boot.trn_boot import _ntff_profile_via_ctypes
        m.set_axon_ntff_profile_hook(
            _ntff_profile_via_ctypes("/opt/axon/libaxon_pjrt.so"))
    except Exception:
        pass


def kernel(X, freqs_cos, freqs_sin, Wq, Wk, Wv, Wo, _trace=False):
    from concourse.bass_utils import run_bass_kernel_spmd

    if _trace:
        _install_ntff_hook()

    X = np.asarray(X, np.float32)
    in_maps = _host_inputs(
        X, np.asarray(freqs_cos, np.float32), np.asarray(freqs_sin, np.float32),
        np.asarray(Wq, np.float32), np.asarray(Wk, np.float32),
        np.asarray(Wv, np.float32), np.asarray(Wo, np.float32),
    )
    nc = get_nc()
    res = run_bass_kernel_spmd(nc, in_maps, core_ids=list(range(8)), trace=_trace)
    out = np.zeros((2, S, DIM), np.float32)
    for c in range(8):
        out[c // 4] += res.results[c]["outp"].astype(np.float32)
    if _trace:
        kernel.last_result = res
    return out


# revision 26
# speedup vs baseline: 1.1748x; 1.1748x over previous
"""Trainium2 Bass kernel for GQA attention (b=2, s=2048, dim=1024, 16 q / 4 kv heads).

Sharding: 8 cores = 2 (batch) x 4 (head groups). Each core owns one batch
element and 4 q-heads + 1 kv-head (Wq/Wk/Wv column-sharded, Wo row-sharded).
Host sums the 4 Wo partials per batch element.

On-device dataflow (per core), everything contraction-major:
  1. qkvT = Wqkv_g @ X^T          (bf16 matmuls, fp32 PSUM)
  2. RoPE on qT/kT via a pair-swap permutation matmul + DVE elementwise
  3. attention with transposed scores S^T[tk, tq] = K_roped @ Q_roped^T:
     exp on ScalarE (scale=1/8 folded in), softmax denominator Z via packed
     M=1 ones-matmuls, P@V with V as the stationary operand (col-packed per
     head pair), normalization by 1/Z applied via gpsimd partition_broadcast
     + DVE multiply on the small attn^T output.
  4. O_partial = attn_g^T.T @ Wo_g^T   -> DRAM fp32
"""

import sys
from contextlib import ExitStack

for _p in ("/opt/trn_rl_repo",):
    if _p not in sys.path:
        sys.path.insert(0, _p)

import numpy as np
import ml_dtypes

BF16 = ml_dtypes.bfloat16

P = 128
S = 2048          # sequence length
DIM = 1024        # model dim
HD = 64           # head dim
NT = S // P       # 16 token tiles
N_CT = DIM // P   # 8 contraction tiles for qkv proj
QKV = 384         # per-core projection rows: 256 q + 64 k + 64 v
PHW = 1024        # tq phase width
NPH = S // PHW    # 2 phases

_NC_CACHE = {}


def _build_kernel_program():
    import concourse.bass as bass
    import concourse.tile as tile
    from concourse import bacc, mybir

    dt = mybir.dt
    f32, bf16 = dt.float32, dt.bfloat16
    AF = mybir.ActivationFunctionType

    nc = bacc.Bacc("TRN2", target_bir_lowering=False, debug=False)

    xt = nc.dram_tensor("xt", [DIM, S], bf16, kind="ExternalInput").ap()
    wqkv = nc.dram_tensor("wqkv", [DIM, QKV], bf16, kind="ExternalInput").ap()
    wo = nc.dram_tensor("wo", [256, DIM], bf16, kind="ExternalInput").ap()
    cost = nc.dram_tensor("cost", [P, S], bf16, kind="ExternalInput").ap()
    sint = nc.dram_tensor("sint", [P, S], bf16, kind="ExternalInput").ap()
    rott = nc.dram_tensor("rott", [P, P], bf16, kind="ExternalInput").ap()
    ident = nc.dram_tensor("ident", [P, P], bf16, kind="ExternalInput").ap()
    utm = nc.dram_tensor("utm", [P, P], bf16, kind="ExternalInput").ap()
    identf = nc.dram_tensor("identf", [P, P], f32, kind="ExternalInput").ap()
    ida = nc.dram_tensor("ida", [P, P], bf16, kind="ExternalInput").ap()
    outp = nc.dram_tensor("outp", [S, DIM], bf16, kind="ExternalOutput").ap()

    with tile.TileContext(nc) as tc:
        with ExitStack() as ctx:
            _emit(ctx, tc, nc, mybir, bass, dict(
                xt=xt, wqkv=wqkv, wo=wo, cost=cost, sint=sint, rott=rott,
                ident=ident, utm=utm, identf=identf, ida=ida, outp=outp,
            ), f32, bf16, AF)
    nc.compile()
    return nc


def _emit(ctx, tc, nc, mybir, bass, io, f32, bf16, AF):
    tp = tc.tile_pool

    const = ctx.enter_context(tp(name="const", bufs=1))
    persist = ctx.enter_context(tp(name="persist", bufs=1))
    tmp = ctx.enter_context(tp(name="tmp", bufs=4))
    ptp = ctx.enter_context(tp(name="pt", bufs=8))
    big = ctx.enter_context(tp(name="big", bufs=4, space="PSUM"))      # [128,512] slots
    pvp = ctx.enter_context(tp(name="pv", bufs=4, space="PSUM"))       # [128,512] slots

    # ---- weights + activations in (issue order = need order) ----
    wqkv_sb = persist.tile([P, N_CT, QKV], bf16, name="wqkv_sb", tag="wqkv_sb")
    nc.sync.dma_start(wqkv_sb[:], io["wqkv"].rearrange("(a p) d -> p a d", p=P))
    xt_sb = persist.tile([P, N_CT, S], bf16, name="xt_sb", tag="xt_sb")
    xt_r = io["xt"].rearrange("(a p) t -> p a t", p=P)
    for a in range(N_CT):
        nc.sync.dma_start(xt_sb[:, a, :], xt_r[:, a, :])
    rott_sb = const.tile([P, P], bf16, tag="rott")
    nc.sync.dma_start(rott_sb[:], io["rott"])
    ident_sb = const.tile([P, P], bf16, tag="ident")
    nc.sync.dma_start(ident_sb[:], io["ident"])
    utm_sb = const.tile([P, P], bf16, tag="utm")
    nc.sync.dma_start(utm_sb[:], io["utm"])
    identf_sb = const.tile([P, P], f32, tag="identf")
    nc.sync.dma_start(identf_sb[:], io["identf"])
    ida_sb = const.tile([P, P], bf16, tag="ida")
    nc.sync.dma_start(ida_sb[:], io["ida"])
    cost_sb = persist.tile([P, S], bf16, name="cost_sb", tag="cost_sb")
    nc.sync.dma_start(cost_sb[:], io["cost"])
    sint_sb = persist.tile([P, S], bf16, name="sint_sb", tag="sint_sb")
    nc.sync.dma_start(sint_sb[:], io["sint"])
    wo_sb = persist.tile([P, 2, DIM], bf16, name="wo_sb", tag="wo_sb")
    nc.sync.dma_start(wo_sb[:], io["wo"].rearrange("(a p) e -> p a e", p=P))

    # ---- phase 1: qkv projection ----
    # qkvT tiles (bf16): q01T [128,S] (heads 0,1), q23T (heads 2,3),
    # kvT [128,S] (rows 0:64 kT, 64:128 vT)
    q01T = persist.tile([P, S], bf16, name="q01T", tag="q01T")
    q23T = persist.tile([P, S], bf16, name="q23T", tag="q23T")
    kvT = persist.tile([P, S], bf16, name="kvT", tag="kvT")
    qkv_dst = [q01T, q23T, kvT]
    for mt in (0, 2):
        for n in range(S // 512):
            ps = big.tile([P, 512], f32, name="ps", tag="big")
            for cti in range(N_CT):
                nc.tensor.matmul(
                    ps,
                    wqkv_sb[:, cti, mt * P:(mt + 1) * P],
                    xt_sb[:, cti, n * 512:(n + 1) * 512],
                    start=(cti == 0), stop=(cti == N_CT - 1),
                )
            nc.scalar.copy(qkv_dst[mt][:, n * 512:(n + 1) * 512], ps)

    # ---- phase 2: RoPE (on q heads and k) ----
    # roped out: q01r, q23r bf16; krep rows 0:64 (k roped), rows 64:128 copy
    q01r = persist.tile([P, S], bf16, name="q01r", tag="q01r")
    q23r = persist.tile([P, S], bf16, name="q23r", tag="q23r")
    krep = persist.tile([P, S], bf16, name="krep", tag="krep")
    rope_jobs = [(q01T, q01r, P), (kvT, krep, 64)]
    for src, dst, rows in rope_jobs:
        for n in range(S // 512):
            sl = slice(n * 512, (n + 1) * 512)
            psr = big.tile([P, 512], f32, name="psr", tag="big")[:rows, :]
            nc.tensor.matmul(
                psr, rott_sb[:rows, :rows], src[:rows, sl],
                start=True, stop=True,
            )
            t1 = tmp.tile([P, 512], bf16, name="ropet1", tag="rope")[:rows]
            nc.vector.tensor_mul(t1, src[:rows, sl], cost_sb[:rows, sl])
            t2 = tmp.tile([P, 512], bf16, name="ropet2", tag="rope")[:rows]
            nc.vector.tensor_mul(t2, psr, sint_sb[:rows, sl])
            nc.vector.tensor_add(dst[:rows, sl], t1, t2)
    # replicate roped k to partitions 64:128 for row-packed score matmuls
    nc.sync.dma_start(krep[64:128, :], krep[0:64, :])

    # ---- phase 3: V = (vT)^T, tiled [tk-tile][128, 64] ----
    v_sb = persist.tile([P, NT, HD + 1], bf16, name="v_sb", tag="v_sb")
    nc.vector.memset(v_sb[:, :, 0:1], 1.0)

    def fill_v_tile(j):
        pst = big.tile([P, 1024], bf16, name="pst", tag="big")[:, :HD]
        nc.tensor.transpose(pst, kvT[64:128, j * P:(j + 1) * P], ident_sb[64:128, 0:HD])
        nc.scalar.copy(v_sb[:, j, 1:HD + 1], pst)

    for j in range(NT // 2):
        fill_v_tile(j)

    # ---- phase 4: attention ----
    # attnT: [4*64 d-rows, S]: two tiles (heads 0,1 | heads 2,3)
    attnT = [persist.tile([P, S], bf16, name="attnT01", tag="attnT01"),
             persist.tile([P, S], bf16, name="attnT23", tag="attnT23")]
    qrs = [q01r, q23r]

    fill_state = {}

    def fill_q23_proj(j):
        # deferred q23 projection, two j-steps per 512-chunk
        n, half = j // 2, j % 2
        if n >= S // 512:
            return
        if half == 0:
            fill_state[n] = big.tile([P, 512], f32, name="psq23", tag="big")
        psn = fill_state[n]
        for cti in (range(0, 4) if half == 0 else range(4, 8)):
            nc.tensor.matmul(
                psn, wqkv_sb[:, cti, P:2 * P],
                xt_sb[:, cti, n * 512:(n + 1) * 512],
                start=(cti == 0), stop=(cti == N_CT - 1),
            )
        if half == 1:
            nc.scalar.copy(q23T[:, n * 512:(n + 1) * 512], psn)
            del fill_state[n]

    def fill_rope_q23():
        for n in range(S // 512):
            sl = slice(n * 512, (n + 1) * 512)
            psr = big.tile([P, 512], f32, name="psrf", tag="big")
            nc.tensor.matmul(psr, rott_sb[:], q23T[:, sl], start=True, stop=True)
            t1 = tmp.tile([P, 512], bf16, name="ropet1", tag="rope")
            nc.vector.tensor_mul(t1, q23T[:, sl], cost_sb[:, sl])
            t2 = tmp.tile([P, 512], bf16, name="ropet2", tag="rope")
            nc.vector.tensor_mul(t2, psr, sint_sb[:, sl])
            nc.vector.tensor_add(q23r[:, sl], t1, t2)

    def emit_wo(tt, e):
        if e == 0:
            fill_state["osb"] = tmp.tile([P, DIM], bf16, name="osb", tag="osb", bufs=2)
        osb = fill_state["osb"]
        po = big.tile([P, 512], f32, name="po", tag="big")
        nc.tensor.matmul(
            po, attnT[0][:, tt * P:(tt + 1) * P], wo_sb[:, 0, e * 512:(e + 1) * 512],
            start=True, stop=False,
        )
        nc.tensor.matmul(
            po, attnT[1][:, tt * P:(tt + 1) * P], wo_sb[:, 1, e * 512:(e + 1) * 512],
            start=False, stop=True,
        )
        nc.vector.tensor_copy(osb[:, e * 512:(e + 1) * 512], po)
        if e == 1:
            nc.sync.dma_start(io["outp"][tt * P:(tt + 1) * P, :], osb[:])

    for ph in range(NPH):
        base = ph * PHW
        jmax = (base + PHW) // P - 1  # last tk tile in this phase
        for hp in range(2):
            # per-chunk PV accumulators; rows 0:64 attn^T, row 64 = Z (V ones col)
            pvE = [pvp.tile([P, 512], f32, name="pvE", tag="pv") for _ in range(2)]
            pvO = [pvp.tile([P, 512], f32, name="pvO", tag="pv") for _ in range(2)]
            for j in range(jmax + 1):
                ssl = max(0, j * P - base)    # phase-local valid tq start
                # --- scores S^T pair + exp, per 512-wide piece ---
                ptA = ptp.tile([P, PHW], bf16, name="ptA", tag="pt")
                ptB = ptp.tile([P, PHW], bf16, name="ptB", tag="pt")
                pieces = []
                if ssl < 512:
                    pieces.append((ssl, 512))
                pieces.append((max(512, ssl), PHW))
                for (lo, hi) in pieces:
                    if lo >= hi:
                        continue
                    w = hi - lo
                    sA = big.tile([P, 512], f32, name="sA", tag="big")[:, :w]
                    sB = big.tile([P, 512], f32, name="sB", tag="big")[:, :w]
                    nc.tensor.matmul(
                        sA,
                        krep[0:64, j * P:(j + 1) * P],
                        qrs[hp][0:64, base + lo:base + hi],
                        start=True, stop=True, tile_position=(0, 0),
                    )
                    nc.tensor.matmul(
                        sB,
                        krep[64:128, j * P:(j + 1) * P],
                        qrs[hp][64:128, base + lo:base + hi],
                        start=True, stop=True, tile_position=(64, 0),
                    )
                    nc.scalar.activation(ptA[:, lo:hi], sA, AF.Exp, scale=0.125)
                    nc.scalar.activation(ptB[:, lo:hi], sB, AF.Exp, scale=0.125)
                # --- causal mask on the diagonal 128-block ---
                if j * P >= base:
                    dl = slice(ssl, ssl + P)
                    nc.vector.tensor_mul(ptA[:, dl], ptA[:, dl], utm_sb[:])
                    nc.vector.tensor_mul(ptB[:, dl], ptB[:, dl], utm_sb[:])
                # --- PV (fused denominator row); evict chunk when complete ---
                for cl in range(2):
                    lo = max(512 * cl, ssl)
                    hi = 512 * (cl + 1)
                    if lo >= hi:
                        continue
                    c_glob = 2 * ph + cl
                    stop_j = min(4 * c_glob + 3, jmax)
                    st = (j == 0)
                    sp = (j == stop_j)
                    co = slice(lo - 512 * cl, hi - 512 * cl)
                    nc.tensor.matmul(
                        pvE[cl][0:HD + 1, co], v_sb[:, j, :], ptA[:, lo:hi],
                        start=st, stop=sp,
                    )
                    nc.tensor.matmul(
                        pvO[cl][0:HD + 1, co], v_sb[:, j, :], ptB[:, lo:hi],
                        start=st, stop=sp,
                    )
                    if sp:
                        # chunk complete: transpose-normalize-transpose-back.
                        gcol = base + 512 * cl
                        pvs_sb = tmp.tile([P, 2, 512], f32, name="pvs", tag="pvs", bufs=2)
                        rec4 = tmp.tile([P, 2, 4], f32, name="rec4", tag="rec4", bufs=2)
                        anrm = tmp.tile([P, 2, 4, HD], bf16, name="anrm", tag="anrm", bufs=2)
                        for h, pv in ((0, pvE[cl]), (1, pvO[cl])):
                            nc.vector.tensor_copy(pvs_sb[0:HD + 1, h, :], pv[0:HD + 1, :])
                            pT = pvp.tile([P, 512], f32, name="pT", tag="pv")[:, 0:260]
                            for b in range(4):
                                nc.tensor.transpose(
                                    pT[:, 65 * b:65 * (b + 1)],
                                    pvs_sb[0:HD + 1, h, 128 * b:128 * (b + 1)],
                                    identf_sb[0:HD + 1, 0:HD + 1])
                            pT3 = pT.rearrange("p (b c) -> p b c", c=65)
                            nc.vector.reciprocal(rec4[:, h, :], pT3[:, :, 0])
                            nc.vector.tensor_mul(
                                anrm[:, h], pT3[:, :, 1:HD + 1],
                                rec4[:, h, :, None].broadcast_to([P, 4, HD]))
                        pout = pvp.tile([P, 512], bf16, name="pout", tag="pv")
                        for h in range(2):
                            tpos = (0, 0) if h == 0 else (0, HD)
                            rows = slice(0, HD) if h == 0 else slice(HD, P)
                            for b in range(4):
                                nc.tensor.transpose(
                                    pout[rows, 128 * b:128 * (b + 1)],
                                    anrm[:, h, b, :], ida_sb[:, :],
                                    tile_position=tpos)
                        nc.vector.tensor_copy(attnT[hp][:, gcol:gcol + 512], pout[:, 0:512])
                # interleaved filler work keeps the PE dense (clock stays warm)
                if ph == 0 and hp == 0:
                    fill_q23_proj(j)
                elif ph == 0 and hp == 1:
                    fill_v_tile(NT // 2 + j)
                elif ph == 1 and hp == 0:
                    emit_wo(j // 2, j % 2)
                elif ph == 1 and hp == 1 and j >= 12:
                    emit_wo(8 + (j - 12), 0)
                    emit_wo(8 + (j - 12), 1)
            if ph == 0 and hp == 0:
                fill_rope_q23()

    # ---- phase 5: remaining output projection tiles ----
    for tt in range(12, NT):
        for e in range(2):
            emit_wo(tt, e)


def _host_inputs(X, cos, sin, Wq, Wk, Wv, Wo):
    """Build the 8 per-core input maps (host-side sharding + layout prep)."""
    cosT = np.ascontiguousarray(cos.T)  # [64, 2048]
    sinT = np.ascontiguousarray(sin.T)
    cost = np.concatenate([cosT, cosT], 0).astype(BF16)  # [128, 2048]
    sint = np.concatenate([sinT, sinT], 0).astype(BF16)
    rott = np.zeros((P, P), np.float32)
    idx = np.arange(0, P, 2)
    rott[idx, idx + 1] = 1.0    # RT[2i, 2i+1] = +1
    rott[idx + 1, idx] = -1.0   # RT[2i+1, 2i] = -1
    rott = rott.astype(BF16)
    ident = np.zeros((P, P), np.float32)
    ident[0:64, 0:64] = np.eye(64)
    ident[64:128, 0:64] = np.eye(64)   # same I64 available at base partition 64
    ident = ident.astype(BF16)
    utm = np.triu(np.ones((P, P), np.float32)).astype(BF16)
    identf = np.eye(P, dtype=np.float32)
    ida = np.eye(P, dtype=np.float32).astype(BF16)

    xts = [np.ascontiguousarray(X[b].T).astype(BF16) for b in range(X.shape[0])]

    in_maps = []
    for c in range(8):
        b, g = c // 4, c % 4
        wqkv = np.concatenate(
            [Wq[256 * g:256 * (g + 1)], Wk[64 * g:64 * (g + 1)], Wv[64 * g:64 * (g + 1)]], 0
        ).T.astype(BF16)                                   # [1024, 384]
        wog = np.ascontiguousarray(Wo[:, 256 * g:256 * (g + 1)].T).astype(BF16)  # [256, 1024]
        in_maps.append({
            "xt": xts[b], "wqkv": np.ascontiguousarray(wqkv), "wo": wog,
            "cost": cost, "sint": sint, "rott": rott, "ident": ident,
            "utm": utm, "identf": identf, "ida": ida,
        })
    return in_maps


def get_nc():
    if "nc" not in _NC_CACHE:
        _NC_CACHE["nc"] = _build_kernel_program()
    return _NC_CACHE["nc"]


def _install_ntff_hook():
    """The agent image's antenv lacks axon_hooks; recreate it so trace=True
    can reach the terminal's NRT profiler (timing only, not needed for
    correctness)."""
    import types
    if "antenv.axon_hooks" in sys.modules:
        return
    try:
        import antenv
        m = types.ModuleType("antenv.axon_hooks")
        holder = {"v": None}
        m.set_axon_ntff_profile_hook = lambda h: holder.__setitem__("v", h)
        m.get_axon_ntff_profile_hook = lambda: holder["v"]
        sys.modules["antenv.axon_hooks"] = m
        antenv.axon_hooks = m
        from trn_agent_boot.trn_boot import _ntff_profile_via_ctypes
        m.set_axon_ntff_profile_hook(
            _ntff_profile_via_ctypes("/opt/axon/libaxon_pjrt.so"))
    except Exception:
        pass


def kernel(X, freqs_cos, freqs_sin, Wq, Wk, Wv, Wo, _trace=False):
    from concourse.bass_utils import run_bass_kernel_spmd

    if _trace:
        _install_ntff_hook()

    X = np.asarray(X, np.float32)
    in_maps = _host_inputs(
        X, np.asarray(freqs_cos, np.float32), np.asarray(freqs_sin, np.float32),
        np.asarray(Wq, np.float32), np.asarray(Wk, np.float32),
        np.asarray(Wv, np.float32), np.asarray(Wo, np.float32),
    )
    nc = get_nc()
    res = run_bass_kernel_spmd(nc, in_maps, core_ids=list(range(8)), trace=_trace)
    out = np.zeros((2, S, DIM), np.float32)
    for c in range(8):
        out[c // 4] += res.results[c]["outp"].astype(np.float32)
    if _trace:
        kernel.last_result = res
    return out
